# revision 2
# baseline (speedup 1.0000x reference)
"""Trainium2 Bass kernel v2 for llama-style GQA causal attention
(B=4, S=1024, D=4096, 32 Q heads / 8 KV heads, head_dim=128, RoPE).

Sharding: 8 cores = 4 batches x 2 head-halves (tensor-parallel over heads).
Core c handles batch b=c//2 and head-half g=c%2 (16 Q heads, 4 KV heads).
Each core computes a partial y^T = (attn_heads @ wo_half)^T in [D, S] layout;
the host sums the two head-half partials per batch and transposes back.

v2 changes vs baseline:
  - all activations SBUF-resident in bf16 (x, q, k, v, o); no DRAM q spill
  - weights streamed in bf16 with host-pretransposed contiguous tile layouts
  - full 32-deep PSUM accumulation for projections (no SBUF re-accumulation)
  - bf16 matmuls everywhere (same PE rate, no fp32r <256-free-dim penalty)
  - fused phase B: per head q-proj -> rope -> previous head's attention so
    the tensor queue stays continuously fed and ACT exp latency hides under
    projection matmuls
"""

import numpy as np
import ml_dtypes

import concourse.bacc as bacc
import concourse.mybir as mybir
import concourse.tile as tile
import concourse.bass_isa as bass_isa
from concourse.bass_utils import run_bass_kernel_spmd

# problem shape (hardcoded per contract)
B, S, D = 4, 1024, 4096
NH, NKV, HD = 32, 8, 128
P = 128
G2 = 2                      # head-halves (TP degree per batch)
QH = NH // G2               # 16 q heads per core
KVH = NKV // G2             # 4 kv heads per core
QD, KVD = QH * HD, KVH * HD # 2048, 512
THETA = 10000.0
SCALE = float(1.0 / np.sqrt(HD))
NEG = -30000.0

NKT = D // P                # 32 k-tiles over the model dim
TC = 512                    # token chunk (matmul free dim)
NTC = S // TC               # 2
NTOK = S // P               # 8 token tiles
NYG = D // (2 * P)          # 16 output m-pair groups
NWOT = QD // P              # 16 wo k-tiles

F32 = mybir.dt.float32
F32R = mybir.dt.float32r
BF16 = mybir.dt.bfloat16

_CACHE = {}


def _body(nc, tc_, io):
    xt, wq, wk, wv, wo, swp, cosf, sinf, maskt, yt = io
    ts = lambda i, n: slice(i * n, (i + 1) * n)

    with (
        tc_.tile_pool(name="const", bufs=1) as cp,
        tc_.tile_pool(name="big", bufs=1) as bigp,
    ):
        # SBUF residents
        x_sb = bigp.tile([P, NKT, S], BF16)    # 64KB/part
        acc_k = bigp.tile([P, KVH, S], BF16)   # 8KB/part  (k^T, rope applied)
        acc_v = bigp.tile([P, NTOK, KVD], BF16)  # 8KB/part (v, token-major)
        acc_q = bigp.tile([P, QH, S], BF16)    # 32KB/part (q^T, rope applied)
        acc_o = bigp.tile([P, QH, S], BF16)    # 32KB/part (o^T, normalized)

        xt_r = xt.ap()  # [NKT, P, S]
        # split x over two DMA queues so the first k-sweep is never DMA-paced
        for kt in range(NKT):
            eng = nc.scalar if kt % 2 == 0 else nc.gpsimd
            eng.dma_start(x_sb[:, kt], xt_r[kt])

        # consts ride the gpsimd queue behind the odd x tiles (needed later)
        swp_sb = cp.tile([P, P], BF16)
        nc.gpsimd.dma_start(swp_sb, swp.ap())
        mask_sb = cp.tile([P, P], F32)
        nc.gpsimd.dma_start(mask_sb, maskt.ap())
        cos_sb = cp.tile([P, S], BF16)
        nc.gpsimd.dma_start(cos_sb, cosf.ap())
        sin_sb = cp.tile([P, S], F32)
        nc.gpsimd.dma_start(sin_sb, sinf.ap())

        # rope: dst_bf16 = raw*cos + swap(raw)*sin, swap via pair-swap matmul
        def rope_apply(psR, p_tmp, raw, dst_slice, t):
            ps_sw = psR.tile([P, TC], F32, tag="sw", name="ps_sw")
            nc.tensor.matmul(ps_sw, swp_sb, raw, start=True, stop=True)
            tmp = p_tmp.tile([P, TC], BF16, tag="rtmp", name="rtmp")
            nc.vector.tensor_mul(tmp, ps_sw, sin_sb[:, ts(t, TC)])
            qc = p_tmp.tile([P, TC], BF16, tag="rqc", name="rqc")
            nc.gpsimd.tensor_mul(qc, raw, cos_sb[:, ts(t, TC)])
            nc.vector.tensor_add(dst_slice, qc, tmp)

        wq_r = wq.ap()  # [QH, P, NKT*HD]
        wo_r = wo.ap()  # [NYG, P, NWOT*2P]
        wq_tiles = {}
        wo_tiles = {}

        with (
            tc_.tile_pool(name="pBw", bufs=3) as pBw,
            tc_.tile_pool(name="pCw", bufs=2) as pCw,
        ):
            def prefetch_wq(h):
                t = pBw.tile([P, NKT * HD], BF16, tag="wq")
                nc.sync.dma_start(t, wq_r[h])
                wq_tiles[h] = t

            def prefetch_wo(yg):
                t = pCw.tile([P, NWOT * 2 * P], BF16, tag="wo")
                nc.sync.dma_start(t, wo_r[yg])
                wo_tiles[yg] = t

            # ---------------- phase A: k/v projections ----------------
            with (
                tc_.tile_pool(name="pAw", bufs=4) as pAw,
                tc_.tile_pool(name="pAr", bufs=2) as pAr,
                tc_.tile_pool(name="pAt", bufs=2) as pAt,
                tc_.tile_pool(name="psA", bufs=6, space="PSUM") as psA,
                tc_.tile_pool(name="psR", bufs=1, space="PSUM") as psR,
            ):
                wk_r = wk.ap()  # [2, NKT, P, 2P] (host pre-split by mg)
                wv_r = wv.ap()

                # k: 2 groups of (2 kv heads x 2 token-chunks); k^T layout
                for mg in range(KVH // 2):
                    ps = [psA.tile([P, TC], F32, tag="g", name=f"psk{_i}")
                          for _i in range(4)]
                    for kt in range(NKT):
                        w_t = pAw.tile([P, 2 * P], BF16, tag="wk")
                        nc.sync.dma_start(w_t, wk_r[mg, kt])
                        for i in range(2):
                            for t in range(NTC):
                                nc.tensor.matmul(
                                    ps[2 * i + t], w_t[:, ts(i, P)],
                                    x_sb[:, kt, ts(t, TC)],
                                    start=(kt == 0), stop=(kt == NKT - 1))
                    # drain + rope (alternate drain engines to avoid ACT pileup)
                    for i in range(2):
                        h = mg * 2 + i
                        for t in range(NTC):
                            raw = pAr.tile([P, TC], BF16, tag="kraw", name="kraw")
                            if t == 0:
                                nc.scalar.activation(
                                    raw, ps[2 * i + t],
                                    mybir.ActivationFunctionType.Copy)
                            else:
                                nc.vector.tensor_copy(raw, ps[2 * i + t])
                            rope_apply(psR, pAt, raw,
                                       acc_k[:, h, ts(t, TC)], t)

                # first q weights, queued behind the k sweep
                prefetch_wq(0)
                prefetch_wq(1)

                # v: 2 groups of 4 token-tiles, [P, KVD] psum each
                for tg in range(NTOK // 4):
                    ps = [psA.tile([P, KVD], F32, tag="g", name=f"psv{_i}")
                          for _i in range(4)]
                    for kt in range(NKT):
                        w_t = pAw.tile([P, KVD], BF16, tag="wv")
                        nc.sync.dma_start(w_t, wv_r[kt])
                        for tm in range(4):
                            nc.tensor.matmul(
                                ps[tm], x_sb[:, kt, ts(tg * 4 + tm, P)], w_t,
                                start=(kt == 0), stop=(kt == NKT - 1))
                    for tm in range(4):
                        if tm % 2 == 0:
                            nc.scalar.activation(
                                acc_v[:, tg * 4 + tm], ps[tm],
                                mybir.ActivationFunctionType.Copy)
                        else:
                            nc.vector.tensor_copy(acc_v[:, tg * 4 + tm], ps[tm])

            # ------------- phase B: q proj + rope + attention (fused) ------
            with (
                tc_.tile_pool(name="pBr", bufs=2) as pBr,
                tc_.tile_pool(name="pBt", bufs=2) as pBt,
                tc_.tile_pool(name="pBp", bufs=3) as pBp,
                tc_.tile_pool(name="pBn", bufs=1) as pBn,
                tc_.tile_pool(name="pBo", bufs=2) as pBo,
                tc_.tile_pool(name="psQ", bufs=2, space="PSUM") as psQ,
                tc_.tile_pool(name="psR2", bufs=1, space="PSUM") as psR2,
                tc_.tile_pool(name="psS", bufs=4, space="PSUM") as psS,
                tc_.tile_pool(name="psO", bufs=1, space="PSUM") as psO,
            ):
                def qp_thunks(h, ps, t):
                    """One thunk per q-projection matmul for (head h, chunk t)."""
                    def mk(kt):
                        def emit(w_t=wq_tiles[h], kt=kt):
                            nc.tensor.matmul(
                                ps[t], w_t[:, ts(kt, HD)],
                                x_sb[:, kt, ts(t, TC)],
                                start=(kt == 0), stop=(kt == NKT - 1))
                        return emit
                    return [mk(kt) for kt in range(NKT)]

                def drain_q(h, ps, t):
                    raw = pBr.tile([P, TC], BF16, tag="qraw", name="qraw")
                    nc.scalar.activation(raw, ps[t],
                                         mybir.ActivationFunctionType.Copy)
                    return raw

                def rot_q(h, raw, t):
                    rope_apply(psR2, pBt, raw, acc_q[:, h, ts(t, TC)], t)

                def attn(h, t, filler):
                    """Attention for (h, t); pops filler thunks (independent
                    tensor-engine work) after each scores+exp so the exp
                    latency is hidden before the l/pv matmuls need p."""
                    g = h // (QH // KVH)
                    nkc = 4 * (t + 1)
                    nfill = max(1, (len(filler) + nkc - 1) // nkc) if filler else 0
                    ps_o = psO.tile([P, TC], F32, tag="o")
                    l_acc = pBn.tile([P, TC], F32, tag="lacc")
                    for kc in range(nkc):
                        j = kc - 4 * t
                        off = max(0, j) * P
                        w = TC - off
                        ps_s = psS.tile([P, TC], F32, tag="s")
                        nc.tensor.matmul(ps_s[:, :w], acc_k[:, g, ts(kc, P)],
                                         acc_q[:, h, t * TC + off:(t + 1) * TC],
                                         start=True, stop=True)
                        if j >= 0:
                            nc.vector.tensor_add(ps_s[:, :P], ps_s[:, :P],
                                                 mask_sb)
                        p = pBp.tile([P, TC], BF16, tag="p")
                        nc.scalar.activation(p[:, :w], ps_s[:, :w],
                                             mybir.ActivationFunctionType.Exp,
                                             scale=SCALE)
                        for _ in range(nfill):
                            if filler:
                                filler.pop(0)()
                        # softmax denominator on the Pool engine (partition
                        # all-reduce outputs the broadcast key-sum directly)
                        if kc == 0:
                            nc.gpsimd.partition_all_reduce(
                                l_acc, p, P, bass_isa.ReduceOp.add)
                        else:
                            pr = pBn.tile([P, TC], F32, tag="prt")
                            nc.gpsimd.partition_all_reduce(
                                pr[:, :w], p[:, :w], P, bass_isa.ReduceOp.add)
                            nc.vector.tensor_add(l_acc[:, off:],
                                                 l_acc[:, off:], pr[:, :w])
                        nc.tensor.matmul(ps_o[:, off:],
                                         acc_v[:, kc, ts(g, P)], p[:, :w],
                                         start=(kc == 0), stop=(kc == nkc - 1),
                                         skip_group_check=True)
                    while filler:
                        filler.pop(0)()
                    o_raw = pBo.tile([P, TC], BF16, tag="oraw")
                    nc.scalar.activation(o_raw, ps_o,
                                         mybir.ActivationFunctionType.Copy)
                    rl = pBn.tile([P, TC], F32, tag="rl")
                    nc.vector.reciprocal(rl, l_acc)
                    nc.vector.tensor_mul(acc_o[:, h, ts(t, TC)], o_raw, rl)

                # per step h: interleave qproj(h) matmuls into attn(h-1)'s
                # chunk stream; rope rotations ride between blocks.
                rot_pending = None  # (h, raw, t) for rot emitted next block
                for h in range(QH + 1):
                    ps = None
                    if h < QH:
                        if h + 2 < QH:
                            prefetch_wq(h + 2)
                        ps = [psQ.tile([P, TC], F32, tag="q", name=f"psq{_t}")
                              for _t in range(NTC)]
                    for t in range(NTC):
                        fill = qp_thunks(h, ps, t) if h < QH else []
                        if rot_pending is not None:
                            rh, rraw, rt = rot_pending
                            if fill:
                                fill.pop(0)()
                            rot_q(rh, rraw, rt)
                            rot_pending = None
                        if h > 0:
                            attn(h - 1, t, fill)
                        else:
                            for f in fill:
                                f()
                        if h < QH:
                            raw = drain_q(h, ps, t)
                            rot_pending = (h, raw, t)
                    if h == QH - 2:
                        prefetch_wo(0)
                if rot_pending is not None:
                    rh, rraw, rt = rot_pending
                    rot_q(rh, rraw, rt)
                    rot_pending = None

            # ---------------- phase C: wo ----------------
            with (
                tc_.tile_pool(name="pCy", bufs=6) as pCy,
                tc_.tile_pool(name="psY", bufs=8, space="PSUM") as psY,
            ):
                for yg in range(NYG):
                    if yg + 1 < NYG:
                        prefetch_wo(yg + 1)
                    w_t = wo_tiles.pop(yg)
                    ps = [psY.tile([P, TC], F32, tag="y", name=f"psy{_i}")
                          for _i in range(4)]
                    for kt in range(NWOT):
                        for i in range(2):
                            for t in range(NTC):
                                nc.tensor.matmul(
                                    ps[2 * i + t],
                                    w_t[:, kt * 2 * P + i * P:
                                        kt * 2 * P + (i + 1) * P],
                                    acc_o[:, kt, ts(t, TC)],
                                    start=(kt == 0), stop=(kt == NWOT - 1))
                    for i in range(2):
                        for t in range(NTC):
                            y_sb = pCy.tile([P, TC], F32, tag="ysb")
                            if (2 * i + t) % 2 == 0:
                                nc.scalar.activation(
                                    y_sb, ps[2 * i + t],
                                    mybir.ActivationFunctionType.Copy)
                            else:
                                nc.vector.tensor_copy(y_sb, ps[2 * i + t])
                            mt = yg * 2 + i
                            yeng = nc.gpsimd if (2 * i + t) % 2 == 0 else nc.sync
                            yeng.dma_start(yt.ap()[ts(mt, P), ts(t, TC)], y_sb)


def _build(loop_k=0):
    nc = bacc.Bacc("TRN2", target_bir_lowering=False, debug=False)
    xt = nc.dram_tensor("xt", [NKT, P, S], BF16, kind="ExternalInput")
    wq = nc.dram_tensor("wq", [QH, P, NKT * HD], BF16, kind="ExternalInput")
    wk = nc.dram_tensor("wk", [2, NKT, P, 2 * P], BF16, kind="ExternalInput")
    wv = nc.dram_tensor("wv", [NKT, P, KVD], BF16, kind="ExternalInput")
    wo = nc.dram_tensor("wo", [NYG, P, NWOT * 2 * P], BF16, kind="ExternalInput")
    swp = nc.dram_tensor("swp", [P, P], BF16, kind="ExternalInput")
    cosf = nc.dram_tensor("cosf", [P, S], BF16, kind="ExternalInput")
    sinf = nc.dram_tensor("sinf", [P, S], F32, kind="ExternalInput")
    maskt = nc.dram_tensor("maskt", [P, P], F32, kind="ExternalInput")
    yt = nc.dram_tensor("yt", [D, S], F32, kind="ExternalOutput")

    with tile.TileContext(nc) as tc_:
        if loop_k:
            with tc_.For_i(0, loop_k, 1):
                _body(nc, tc_, (xt, wq, wk, wv, wo, swp, cosf, sinf, maskt, yt))
        else:
            _body(nc, tc_, (xt, wq, wk, wv, wo, swp, cosf, sinf, maskt, yt))
    nc.compile()
    return nc


def get_nc():
    if "nc" not in _CACHE:
        _CACHE["nc"] = _build()
    return _CACHE["nc"]


def host_inputs(x, wq, wk, wv, wo):
    """Shard + lay out the full inputs into per-core in_maps."""
    bf = ml_dtypes.bfloat16
    x = np.asarray(x, np.float32)
    wq = np.asarray(wq, np.float32)
    wk = np.asarray(wk, np.float32)
    wv = np.asarray(wv, np.float32)
    wo = np.asarray(wo, np.float32)

    # rope tables in [hd, token] layout, pair-duplicated over partitions
    freqs = 1.0 / (THETA ** (np.arange(0, HD, 2, dtype=np.float32) / HD))
    ang = np.outer(np.arange(S, dtype=np.float32), freqs)  # [S, 64]
    cosf = np.repeat(np.cos(ang), 2, axis=1).T.astype(bf).copy()
    sinf = np.repeat(np.sin(ang), 2, axis=1).T.astype(np.float32).copy()
    sw = np.zeros((P, P), np.float32)
    for i in range(P // 2):
        sw[2 * i, 2 * i + 1] = -1.0
        sw[2 * i + 1, 2 * i] = 1.0
    swp = np.ascontiguousarray(sw.T).astype(bf)

    kp = np.arange(P)[:, None]
    qf = np.arange(P)[None, :]
    maskt = np.where(kp <= qf, 0.0, NEG).astype(np.float32)

    in_maps = []
    for c in range(8):
        b, g = c // G2, c % G2
        # x^T tiles: [NKT, P, S]
        xh = np.ascontiguousarray(
            x[b].T.reshape(NKT, P, S).astype(bf))
        # wq half -> [QH, P, NKT*HD]
        wqh = wq[:, g * QD:(g + 1) * QD].reshape(NKT, P, QH, HD)
        wqh = np.ascontiguousarray(
            wqh.transpose(2, 1, 0, 3).reshape(QH, P, NKT * HD).astype(bf))
        # wk/wv half -> [NKT, P, KVD]
        wkh = np.ascontiguousarray(
            wk[:, g * KVD:(g + 1) * KVD].reshape(NKT, P, 2, 2 * P)
            .transpose(2, 0, 1, 3).astype(bf))
        wvh = np.ascontiguousarray(
            wv[:, g * KVD:(g + 1) * KVD].reshape(NKT, P, KVD).astype(bf))
        # wo half -> [NYG, P, NWOT*2P]
        woh = wo[g * QD:(g + 1) * QD].reshape(NWOT, P, NYG, 2 * P)
        woh = np.ascontiguousarray(
            woh.transpose(2, 1, 0, 3).reshape(NYG, P, NWOT * 2 * P).astype(bf))
        in_maps.append({
            "xt": xh, "wq": wqh, "wk": wkh, "wv": wvh, "wo": woh,
            "swp": swp, "cosf": cosf, "sinf": sinf, "maskt": maskt,
        })
    return in_maps


def kernel(x, wq, wk, wv, wo):
    in_maps = host_inputs(x, wq, wk, wv, wo)
    nc = get_nc()
    res = run_bass_kernel_spmd(nc, in_maps, core_ids=list(range(8)))
    y = np.empty((B, S, D), np.float32)
    for b in range(B):
        y[b] = (res.results[G2 * b]["yt"] + res.results[G2 * b + 1]["yt"]).T
    return y
